# revision 6
# baseline (speedup 1.0000x reference)
"""Trainium2 Bass kernel for nn_Model4 (retrieval_knn).

Model: 3 l2-normalized feature streams -> 4 chained MultiheadAttention blocks
-> full = rt @ t_r.T -> per-group cosine logits [4, 256, 256].

Sharding (8 cores = 4 row-groups x 2 head-groups):
  core c = 2*g + j.  g in 0..3 owns rows R_g = [256g, 256g+256) (== final group g),
  j in 0..1 owns heads {2j, 2j+1} == feature columns [512j, 512j+512) of qkv space.

All activations are kept "feat-major" in SBUF: X.T as [feat(partition), rows(free)]
so every GEMM is a natural matmul without transposes (weights are host-transposed).
Attention uses transposed softmax (scoresT [S, L], no max subtraction -- scores are
~1e-3 magnitude) with column sums done via ones-vector matmuls on the PE.

Per MHA: K/V projections are computed S-sharded and AllGather'd across row-groups
(4-rank groups [[0,2,4,6],[1,3,5,7]]); attention context halves are exchanged
within the (g,*) pair (2-rank groups) before the (replicated) output projection.

Matmuls run in float32r (TF32-like, full PE rate at free-dim >= 256).
"""
import sys

sys.path.insert(0, "/opt/trn_rl_repo")

import numpy as np

import concourse.bass as bass  # noqa: F401
import concourse.tile as tile
import concourse.mybir as mybir
from concourse import bacc
from concourse.bass_utils import run_bass_kernel_spmd

E = 1024
P = 128
KO = E // P          # 8 feature chunks
RG = 256             # rows per group
NCORES = 8
PIECE = P * 4 * RG   # 131072 elements = one [128,4,256] / [128,2,512] piece
F32 = mybir.dt.float32
F32R = mybir.dt.float32r
AF = mybir.ActivationFunctionType
GROUPS4 = [[0, 2, 4, 6], [1, 3, 5, 7]]   # gather S-shards across row-groups
GROUPS2 = [[0, 1], [2, 3], [4, 5], [6, 7]]  # exchange head halves within pair
EPS = 1e-8

_CACHE = {}


def build_nc():
    nc = bacc.Bacc("TRN2", target_bir_lowering=False, debug=False,
                   num_devices=NCORES)
    dram = {}

    def din(name, shape, dt=F32R):
        dram[name] = nc.dram_tensor(name, shape, dt, kind="ExternalInput").ap()

    # raw feature slices (feat-major, this core's 256 rows)
    din("x_text", [E, RG], F32)
    din("x_loc", [E, RG], F32)
    din("x_glob", [E, RG], F32)
    # full (replicated) projection weights, host-transposed to [in, out]
    for w in ("w_tl", "w_tg", "w_rep"):
        din(w, [E, E])
    for b in ("b_tl", "b_tg", "b_rep"):
        din(b, [E], F32)
    din("pos_l", [E], F32)
    din("pos_g", [E], F32)
    # per-MHA weights; q/k/v are this core's head-half [in, 512]
    for m in ("tl", "tg", "ff", "rt"):
        din(f"wq_{m}", [E, 512])
        din(f"wk_{m}", [E, 512])
        din(f"wv_{m}", [E, 512])
        din(f"wo_{m}", [E, E])
        din(f"bq_{m}", [512], F32)
        din(f"bk_{m}", [512], F32)
        din(f"bv_{m}", [512], F32)
        din(f"bo_{m}", [E], F32)

    out_logits = nc.dram_tensor("logits", [RG, RG], F32,
                                kind="ExternalOutput").ap()

    from contextlib import ExitStack
    with tile.TileContext(nc) as tc, ExitStack() as ctx:
            consts = ctx.enter_context(tc.tile_pool(name="consts", bufs=1))
            acts = ctx.enter_context(tc.tile_pool(name="acts", bufs=6))
            pers = ctx.enter_context(tc.tile_pool(name="pers", bufs=1))
            qps = ctx.enter_context(tc.tile_pool(name="qps", bufs=3))
            exps = ctx.enter_context(tc.tile_pool(name="exps", bufs=2))
            kpfp = ctx.enter_context(tc.tile_pool(name="kpfp", bufs=1))
            vpfp = ctx.enter_context(tc.tile_pool(name="vpfp", bufs=1))
            accs = ctx.enter_context(tc.tile_pool(name="accs", bufs=2))
            accfp = ctx.enter_context(tc.tile_pool(name="accfp", bufs=1))
            kvs = ctx.enter_context(tc.tile_pool(name="kvs", bufs=2))
            sqs = ctx.enter_context(tc.tile_pool(name="sqs", bufs=2))
            bcs = ctx.enter_context(tc.tile_pool(name="bcs", bufs=2))
            smalls = ctx.enter_context(tc.tile_pool(name="smalls", bufs=2))
            weights = ctx.enter_context(tc.tile_pool(name="weights", bufs=2))
            outs = ctx.enter_context(tc.tile_pool(name="outs", bufs=1))
            ps256 = ctx.enter_context(tc.tile_pool(name="ps256", bufs=3, space="PSUM"))
            ps512 = ctx.enter_context(tc.tile_pool(name="ps512", bufs=2, space="PSUM"))
            pssum = ctx.enter_context(tc.tile_pool(name="pssum", bufs=2, space="PSUM"))
            dram_p = ctx.enter_context(tc.tile_pool(name="dram_p", bufs=1, space="DRAM"))
            # ---------- constants ----------
            # (memset can't write f32r directly; round through a DVE copy)
            ones_cf = consts.tile([P, 1], F32)
            nc.vector.memset(ones_cf, 1.0)
            ones_col = consts.tile([P, 1], F32R)
            nc.vector.tensor_copy(ones_col, ones_cf)
            ones_rf = consts.tile([1, P], F32)
            nc.vector.memset(ones_rf, 1.0)
            ones_row = consts.tile([1, P], F32R)
            nc.vector.tensor_copy(ones_row, ones_rf)

            def load_bias_pp(name, n):
                """[n] dram -> [128, n//128] per-partition scalar layout."""
                t = consts.tile([P, n // P], F32, name=f"c_{name}")
                nc.sync.dma_start(t, dram[name].rearrange("(c p) -> p c", p=P))
                return t

            bias_pp = {}
            for nm in ("b_tl", "b_tg", "b_rep", "pos_l", "pos_g"):
                bias_pp[nm] = load_bias_pp(nm, E)
            for m in ("tl", "tg", "ff", "rt"):
                bias_pp[f"bq_{m}"] = load_bias_pp(f"bq_{m}", 512)
                bias_pp[f"bk_{m}"] = load_bias_pp(f"bk_{m}", 512)
                bias_pp[f"bv_{m}"] = load_bias_pp(f"bv_{m}", 512)
                bias_pp[f"bo_{m}"] = load_bias_pp(f"bo_{m}", E)

            # ---------- helpers ----------
            def load_w(name, half):
                """weight [1024, 512] (or half of [1024,1024]) -> [128,8,512]."""
                t = weights.tile([P, KO, 512], F32R, tag="w",
                                 name=f"w_{name}_{half}")
                src = dram[name]
                if src.shape[1] == E:
                    src = src[:, half * 512:(half + 1) * 512]
                nc.sync.dma_start(t, src.rearrange("(ko p) c -> p ko c", p=P))
                return t

            def bcast_row(row_f32r, n, out_dt=F32):
                """[1, n] f32r -> [128, n] broadcast via K=1 outer product."""
                if n > 256:
                    ps = ps512.tile([P, n], F32, tag="mm512", name="ps_bc")
                else:
                    ps = ps256.tile([P, n], F32, tag="mm", name="ps_bc")
                nc.tensor.matmul(ps, ones_row, row_f32r, start=True, stop=True)
                out = bcs.tile([P, n], out_dt, tag=f"bc{n}")
                nc.any.tensor_copy(out=out, in_=ps)
                return out

            def gemm_fm(w_tiles, act, out, nco, bias=None, residual=None):
                """Feat-major GEMM: out[:, c, :] = sum_ko w[ko,c-chunk].T@act[ko]
                w_tiles: list of [128, 8, 512] tiles covering nco*128 out chans.
                act [128, 8, R] f32r; out [128, nco, R]; bias [128, nco] f32.
                """
                R = act.shape[2]
                for c in range(nco):
                    w_sb = w_tiles[c // 4]
                    cc = c % 4
                    ps = ps256.tile([P, R], F32, tag="mm")
                    for ko in range(KO):
                        nc.tensor.matmul(ps, w_sb[:, ko, cc * P:(cc + 1) * P],
                                         act[:, ko], start=(ko == 0),
                                         stop=(ko == KO - 1))
                    if bias is not None:
                        nc.vector.tensor_scalar_add(out[:, c], ps,
                                                    bias[:, c:c + 1])
                        if residual is not None:
                            nc.vector.tensor_add(
                                out[:, c], out[:, c].bitcast(F32),
                                residual[:, c])
                    elif residual is not None:
                        nc.vector.tensor_add(out[:, c], ps, residual[:, c])
                    else:
                        nc.any.tensor_copy(out=out[:, c], in_=ps)

            def colsum_inv(src, nko, with_sqrt_eps=False):
                """src [128, nko, R] (read as f32): per-column (free-dim row)
                1/sum of squares (with sqrt) or caller handles; returns
                inv [1, R] f32r."""
                R = src.shape[2]
                ps = pssum.tile([1, R], F32, tag="cs")
                for ko in range(nko):
                    sq = sqs.tile([P, R], F32R, tag="sq")
                    nc.vector.tensor_mul(sq, src[:, ko].bitcast(F32),
                                         src[:, ko].bitcast(F32))
                    nc.tensor.matmul(ps, ones_col, sq, start=(ko == 0),
                                     stop=(ko == nko - 1))
                inv = smalls.tile([1, R], F32R, tag="inv")
                norm = smalls.tile([1, R], F32, tag="nrm")
                nc.scalar.sqrt(norm, ps)
                if with_sqrt_eps:
                    nc.vector.tensor_scalar_max(norm, norm, EPS)
                with nc.allow_low_precision(reason="fp32r rounding intended"):
                    nc.vector.reciprocal(inv, norm)
                return inv

            def attention(qp, kpf, vpf, acc_out, bv_pp):
                """qp [128,4,256] f32r; kpf [128,4(gs),4(dc),256] f32r;
                vpf [128,4(gs),2(sc),512] f32r; acc_out [128,4,256] f32r."""
                for h in range(2):
                    expt = exps.tile([P, KO, RG], F32R, tag="exp",
                                     name=f"expt{h}")
                    pss = pssum.tile([1, RG], F32, tag="cs")
                    for s in range(8):
                        ps = ps256.tile([P, RG], F32, tag="mm")
                        for dk in range(2):
                            nc.tensor.matmul(
                                ps,
                                kpf[:, s // 2, 2 * h + dk,
                                    (s % 2) * P:(s % 2 + 1) * P],
                                qp[:, 2 * h + dk],
                                start=(dk == 0), stop=(dk == 1))
                        nc.scalar.activation(expt[:, s], ps, AF.Exp,
                                             scale=0.0625)
                    for s in range(8):
                        nc.tensor.matmul(pss, ones_col, expt[:, s],
                                         start=(s == 0), stop=(s == 7))
                    inv = smalls.tile([1, RG], F32R, tag="inv")
                    with nc.allow_low_precision(reason="fp32r rounding intended"):
                        nc.vector.reciprocal(inv, pss)
                    bc = bcast_row(inv, RG)
                    for dk in range(2):
                        ps = ps256.tile([P, RG], F32, tag="mm")
                        for s in range(8):
                            nc.tensor.matmul(
                                ps,
                                vpf[:, s // 2, s % 2,
                                    256 * h + P * dk:256 * h + P * (dk + 1)],
                                expt[:, s],
                                start=(s == 0), stop=(s == 7))
                        nc.vector.tensor_mul(acc_out[:, 2 * h + dk], ps, bc)
                        nc.vector.tensor_scalar_add(
                            acc_out[:, 2 * h + dk],
                            acc_out[:, 2 * h + dk].bitcast(F32),
                            bv_pp[:, 2 * h + dk:2 * h + dk + 1])

            def kv_project(m, kv_src):
                """returns (kp [128,4,256] f32r, vp [128,2,512] f32r)."""
                wk = load_w(f"wk_{m}", 0)
                kp = kvs.tile([P, 4, RG], F32R, tag="kp", name=f"kp_{m}")
                gemm_fm([wk], kv_src, kp, 4, bias=bias_pp[f"bk_{m}"])
                wv = load_w(f"wv_{m}", 0)
                vp = kvs.tile([P, 2, 512], F32R, tag="vp", name=f"vp_{m}")
                for mc in range(2):
                    ps = ps512.tile([P, 512], F32, tag="mm512")
                    for ko in range(KO):
                        nc.tensor.matmul(ps, kv_src[:, ko, mc * P:(mc + 1) * P],
                                         wv[:, ko], start=(ko == 0),
                                         stop=(ko == KO - 1))
                    nc.any.tensor_copy(out=vp[:, mc], in_=ps)
                return kp, vp

            def pack_piece(inbuf, off, sb_tile):
                shp = sb_tile.shape
                nc.sync.dma_start(
                    inbuf[off:off + P * shp[1] * shp[2]].rearrange(
                        "(p a b) -> p a b", p=P, a=shp[1]), sb_tile)

            def allgather(inbuf, outbuf, groups):
                nc.gpsimd.collective_compute(
                    "AllGather", mybir.AluOpType.bypass,
                    replica_groups=groups,
                    ins=[inbuf.opt()], outs=[outbuf.opt()])

            def load_kv_full(outbuf, kp_off, vp_off, m):
                kpf = kpfp.tile([P, 4, 4, RG], F32R, tag="kpf",
                                name=f"kpf_{m}")
                vpf = vpfp.tile([P, 4, 2, 512], F32R, tag="vpf",
                                name=f"vpf_{m}")
                for gs in range(4):
                    nc.sync.dma_start(
                        kpf[:, gs],
                        outbuf[gs, kp_off:kp_off + PIECE].rearrange(
                            "(p a b) -> p a b", p=P, a=4))
                    nc.sync.dma_start(
                        vpf[:, gs],
                        outbuf[gs, vp_off:vp_off + PIECE].rearrange(
                            "(p a b) -> p a b", p=P, a=2))
                return kpf, vpf

            def out_proj(m, outbuf2, acc_off, residual, out_tile):
                accf = accfp.tile([P, KO, RG], F32R, tag="accf",
                                  name=f"accf_{m}")
                for pos in range(2):
                    nc.sync.dma_start(
                        accf[:, pos * 4:(pos + 1) * 4],
                        outbuf2[pos, acc_off:acc_off + PIECE].rearrange(
                            "(p a b) -> p a b", p=P, a=4))
                wo = [load_w(f"wo_{m}", 0), load_w(f"wo_{m}", 1)]
                gemm_fm(wo, accf, out_tile, 8, bias=bias_pp[f"bo_{m}"],
                        residual=residual)

            # ---------- stage 0: load + normalize ----------
            def load_raw(name):
                t = acts.tile([P, KO, RG], F32, tag="act", name=f"raw_{name}")
                nc.sync.dma_start(t, dram[name].rearrange(
                    "(ko p) r -> p ko r", p=P))
                return t

            textT = load_raw("x_text")
            locT = load_raw("x_loc")
            globT = load_raw("x_glob")

            def normalize(raw, out, pos_pp=None):
                inv = colsum_inv(raw, KO)
                bc = bcast_row(inv, RG)
                for ko in range(KO):
                    nc.vector.tensor_mul(out[:, ko], raw[:, ko], bc)
                    if pos_pp is not None:
                        nc.vector.tensor_scalar_add(
                            out[:, ko], out[:, ko].bitcast(F32),
                            pos_pp[:, ko:ko + 1])

            textn = acts.tile([P, KO, RG], F32R, tag="act", name="textn")
            normalize(textT, textn)
            localn = pers.tile([P, KO, RG], F32R, name="localn")
            normalize(locT, localn)
            kvl = acts.tile([P, KO, RG], F32R, tag="act", name="kvl")
            for ko in range(KO):
                nc.vector.tensor_scalar_add(kvl[:, ko],
                                            localn[:, ko].bitcast(F32),
                                            bias_pp["pos_l"][:, ko:ko + 1])
            kvg = acts.tile([P, KO, RG], F32R, tag="act", name="kvg")
            normalize(globT, kvg, pos_pp=bias_pp["pos_g"])

            # ---------- stage A: text projections ----------
            qp_tl = qps.tile([P, 4, RG], F32R, tag="qp", name="qp_tl")
            gemm_fm([load_w("wq_tl", 0)], textn, qp_tl, 4,
                    bias=bias_pp["bq_tl"])
            qp_tg = qps.tile([P, 4, RG], F32R, tag="qp", name="qp_tg")
            gemm_fm([load_w("wq_tg", 0)], textn, qp_tg, 4,
                    bias=bias_pp["bq_tg"])
            t_l = acts.tile([P, KO, RG], F32, tag="act", name="t_l")
            gemm_fm([load_w("w_tl", 0), load_w("w_tl", 1)], textn, t_l, 8,
                    bias=bias_pp["b_tl"])
            t_g = acts.tile([P, KO, RG], F32, tag="act", name="t_g")
            gemm_fm([load_w("w_tg", 0), load_w("w_tg", 1)], textn, t_g, 8,
                    bias=bias_pp["b_tg"])
            t_r = acts.tile([P, KO, RG], F32R, tag="act", name="t_r")
            gemm_fm([load_w("w_rep", 0), load_w("w_rep", 1)], textn, t_r, 8,
                    bias=bias_pp["b_rep"])

            # ---------- stage B: tl + tg MHAs ----------
            kp_tl, vp_tl = kv_project("tl", kvl)
            kp_tg, vp_tg = kv_project("tg", kvg)
            in1 = dram_p.tile([4 * PIECE], F32R, name="in1")
            out1 = dram_p.tile([4, 4 * PIECE], F32R, name="out1")
            pack_piece(in1, 0 * PIECE, kp_tl)
            pack_piece(in1, 1 * PIECE, vp_tl)
            pack_piece(in1, 2 * PIECE, kp_tg)
            pack_piece(in1, 3 * PIECE, vp_tg)
            allgather(in1, out1, GROUPS4)

            kpf_tl, vpf_tl = load_kv_full(out1, 0, PIECE, "tl")
            acc_tl = accs.tile([P, 4, RG], F32R, tag="acc", name="acc_tl")
            attention(qp_tl, kpf_tl, vpf_tl, acc_tl, bias_pp["bv_tl"])
            kpf_tg, vpf_tg = load_kv_full(out1, 2 * PIECE, 3 * PIECE, "tg")
            acc_tg = accs.tile([P, 4, RG], F32R, tag="acc", name="acc_tg")
            attention(qp_tg, kpf_tg, vpf_tg, acc_tg, bias_pp["bv_tg"])

            in2 = dram_p.tile([2 * PIECE], F32R, name="in2")
            out2 = dram_p.tile([2, 2 * PIECE], F32R, name="out2")
            pack_piece(in2, 0, acc_tl)
            pack_piece(in2, PIECE, acc_tg)
            allgather(in2, out2, GROUPS2)

            lt = acts.tile([P, KO, RG], F32R, tag="act", name="lt")
            out_proj("tl", out2, 0, t_l, lt)
            gt = acts.tile([P, KO, RG], F32R, tag="act", name="gt")
            out_proj("tg", out2, PIECE, t_g, gt)

            # ---------- stage C: ff MHA (q=lt, kv=gt) ----------
            qp_ff = qps.tile([P, 4, RG], F32R, tag="qp", name="qp_ff")
            gemm_fm([load_w("wq_ff", 0)], lt, qp_ff, 4, bias=bias_pp["bq_ff"])
            kp_ff, vp_ff = kv_project("ff", gt)
            in3 = dram_p.tile([4 * PIECE], F32R, name="in3")
            out3 = dram_p.tile([4, 4 * PIECE], F32R, name="out3")
            pack_piece(in3, 0, kp_ff)
            pack_piece(in3, PIECE, vp_ff)
            pack_piece(in3, 2 * PIECE, t_r)
            allgather(in3, out3, GROUPS4)

            kpf_ff, vpf_ff = load_kv_full(out3, 0, PIECE, "ff")
            acc_ff = accs.tile([P, 4, RG], F32R, tag="acc", name="acc_ff")
            attention(qp_ff, kpf_ff, vpf_ff, acc_ff, bias_pp["bv_ff"])
            in4 = dram_p.tile([PIECE], F32R, name="in4")
            out4 = dram_p.tile([2, PIECE], F32R, name="out4")
            pack_piece(in4, 0, acc_ff)
            allgather(in4, out4, GROUPS2)
            ff = acts.tile([P, KO, RG], F32R, tag="act", name="ff")
            out_proj("ff", out4, 0, lt, ff)

            # ---------- stage D: rt MHA (q=t_r, kv=ff) ----------
            qp_rt = qps.tile([P, 4, RG], F32R, tag="qp", name="qp_rt")
            gemm_fm([load_w("wq_rt", 0)], t_r, qp_rt, 4, bias=bias_pp["bq_rt"])
            kp_rt, vp_rt = kv_project("rt", ff)
            in5 = dram_p.tile([2 * PIECE], F32R, name="in5")
            out5 = dram_p.tile([4, 2 * PIECE], F32R, name="out5")
            pack_piece(in5, 0, kp_rt)
            pack_piece(in5, PIECE, vp_rt)
            allgather(in5, out5, GROUPS4)

            kpf_rt, vpf_rt = load_kv_full(out5, 0, PIECE, "rt")
            acc_rt = accs.tile([P, 4, RG], F32R, tag="acc", name="acc_rt")
            attention(qp_rt, kpf_rt, vpf_rt, acc_rt, bias_pp["bv_rt"])
            in6 = dram_p.tile([PIECE], F32R, name="in6")
            out6 = dram_p.tile([2, PIECE], F32R, name="out6")
            pack_piece(in6, 0, acc_rt)
            allgather(in6, out6, GROUPS2)
            rt = acts.tile([P, KO, RG], F32R, tag="act", name="rt")
            out_proj("rt", out6, 0, None, rt)

            # ---------- stage E: full = rt @ t_r.T, cosine logits ----------
            fullT = acts.tile([P, KO, RG], F32, tag="act", name="fullT")
            for gs in range(4):
                trf = exps.tile([P, KO, RG], F32R, tag="exp", name=f"trf{gs}")
                nc.sync.dma_start(
                    trf, out3[gs, 2 * PIECE:4 * PIECE].rearrange(
                        "(p a b) -> p a b", p=P, a=KO))
                for mh in range(2):
                    mc = gs * 2 + mh
                    ps = ps256.tile([P, RG], F32, tag="mm")
                    for ko in range(KO):
                        nc.tensor.matmul(ps, trf[:, ko, mh * P:(mh + 1) * P],
                                         rt[:, ko], start=(ko == 0),
                                         stop=(ko == KO - 1))
                    nc.any.tensor_copy(out=fullT[:, mc], in_=ps)

            inv_full = colsum_inv(fullT, KO, with_sqrt_eps=True)
            bc_full = bcast_row(inv_full, RG)
            ffn = acts.tile([P, KO, RG], F32R, tag="act", name="ffn")
            for ko in range(KO):
                nc.vector.tensor_mul(ffn[:, ko], fullT[:, ko], bc_full)

            lg = outs.tile([P, 2, RG], F32, name="lg")
            for lc in range(2):
                ps = ps256.tile([P, RG], F32, tag="mm")
                for ko in range(KO):
                    nc.tensor.matmul(ps, ffn[:, ko, lc * P:(lc + 1) * P],
                                     localn[:, ko], start=(ko == 0),
                                     stop=(ko == KO - 1))
                nc.any.tensor_copy(out=lg[:, lc], in_=ps)
            nc.sync.dma_start(
                out_logits.rearrange("(lc p) q -> p lc q", p=P), lg)

    nc.compile()
    return nc


def make_in_maps(local_feat, global_feat, text_feat,
                 w_tl, b_tl, w_tg, b_tg, w_rep, b_rep,
                 pos_local, pos_global, mha_params):
    """mha_params: dict m -> (wi, bi, wo, bo)."""
    f32 = np.float32
    textT = np.ascontiguousarray(text_feat.T.astype(f32))
    locT = np.ascontiguousarray(local_feat.T.astype(f32))
    globT = np.ascontiguousarray(global_feat.T.astype(f32))
    shared = {
        "w_tl": np.ascontiguousarray(w_tl.T.astype(f32)),
        "w_tg": np.ascontiguousarray(w_tg.T.astype(f32)),
        "w_rep": np.ascontiguousarray(w_rep.T.astype(f32)),
        "b_tl": b_tl.astype(f32), "b_tg": b_tg.astype(f32),
        "b_rep": b_rep.astype(f32),
        "pos_l": pos_local.astype(f32), "pos_g": pos_global.astype(f32),
    }
    per_j = {}
    for j in range(2):
        d = {}
        for m, (wi, bi, wo, bo) in mha_params.items():
            sl = slice(512 * j, 512 * (j + 1))
            d[f"wq_{m}"] = np.ascontiguousarray(wi[0 * E:1 * E][sl].T.astype(f32))
            d[f"wk_{m}"] = np.ascontiguousarray(wi[1 * E:2 * E][sl].T.astype(f32))
            d[f"wv_{m}"] = np.ascontiguousarray(wi[2 * E:3 * E][sl].T.astype(f32))
            d[f"wo_{m}"] = np.ascontiguousarray(wo.T.astype(f32))
            d[f"bq_{m}"] = bi[0 * E:1 * E][sl].astype(f32)
            d[f"bk_{m}"] = bi[1 * E:2 * E][sl].astype(f32)
            d[f"bv_{m}"] = bi[2 * E:3 * E][sl].astype(f32)
            d[f"bo_{m}"] = bo.astype(f32)
        per_j[j] = d

    in_maps = []
    for c in range(NCORES):
        g, j = c // 2, c % 2
        rs = slice(RG * g, RG * (g + 1))
        m = {
            "x_text": np.ascontiguousarray(textT[:, rs]),
            "x_loc": np.ascontiguousarray(locT[:, rs]),
            "x_glob": np.ascontiguousarray(globT[:, rs]),
        }
        m.update(shared)
        m.update(per_j[j])
        in_maps.append(m)
    return in_maps


def kernel(local_feat, global_feat, text_feat,
           w_tl, b_tl, w_tg, b_tg, w_rep, b_rep,
           pos_local, pos_global,
           tl_wi, tl_bi, tl_wo, tl_bo,
           tg_wi, tg_bi, tg_wo, tg_bo,
           ff_wi, ff_bi, ff_wo, ff_bo,
           rt_wi, rt_bi, rt_wo, rt_bo,
           n_groups):
    assert int(n_groups) == 4
    if "nc" not in _CACHE:
        _CACHE["nc"] = build_nc()
    nc = _CACHE["nc"]
    mha_params = {
        "tl": (tl_wi, tl_bi, tl_wo, tl_bo),
        "tg": (tg_wi, tg_bi, tg_wo, tg_bo),
        "ff": (ff_wi, ff_bi, ff_wo, ff_bo),
        "rt": (rt_wi, rt_bi, rt_wo, rt_bo),
    }
    in_maps = make_in_maps(np.asarray(local_feat), np.asarray(global_feat),
                           np.asarray(text_feat),
                           np.asarray(w_tl), np.asarray(b_tl),
                           np.asarray(w_tg), np.asarray(b_tg),
                           np.asarray(w_rep), np.asarray(b_rep),
                           np.asarray(pos_local), np.asarray(pos_global),
                           {k: tuple(np.asarray(x) for x in v)
                            for k, v in mha_params.items()})
    res = run_bass_kernel_spmd(nc, in_maps, core_ids=list(range(NCORES)))
    _CACHE["last_results"] = res
    out = np.empty((4, RG, RG), dtype=np.float32)
    for g in range(4):
        out[g] = res.results[2 * g]["logits"]
    return out


# revision 13
# speedup vs baseline: 1.2300x; 1.2300x over previous
"""Trainium2 Bass kernel for nn_Model4 (retrieval_knn).

Model: 3 l2-normalized feature streams -> 4 chained MultiheadAttention blocks
-> full = rt @ t_r.T -> per-group cosine logits [4, 256, 256].

Sharding (8 cores = 4 row-groups x 2 head-groups):
  core c = 2*g + j.  g in 0..3 owns rows R_g = [256g, 256g+256) (== final group g),
  j in 0..1 owns heads {2j, 2j+1} == feature columns [512j, 512j+512) of qkv space.

All activations are kept "feat-major" in SBUF: X.T as [feat(partition), rows(free)]
so every GEMM is a natural matmul without transposes (weights are host-transposed).
Attention uses transposed softmax (scoresT [S, L], no max subtraction -- scores are
~1e-3 magnitude) with column sums done via ones-vector matmuls on the PE.

Per MHA: K/V projections are computed S-sharded and AllGather'd across row-groups
(4-rank groups [[0,2,4,6],[1,3,5,7]]); attention context halves are exchanged
within the (g,*) pair (2-rank groups) before the (replicated) output projection.

Precision: weights + attention path in bf16 (fp32 PSUM accumulate); the l2-norm
statistics and final cosine/logits path stay in fp32(r).
"""
import sys

sys.path.insert(0, "/opt/trn_rl_repo")

import ml_dtypes
import numpy as np

import concourse.bass as bass  # noqa: F401
import concourse.tile as tile
import concourse.mybir as mybir
from concourse import bacc
from concourse.bass_utils import run_bass_kernel_spmd

E = 1024
P = 128
KO = E // P          # 8 feature chunks
RG = 256             # rows per group
NCORES = 8
PIECE = P * 4 * RG   # 131072 elements: [128,4,256] / [128,2,512] piece
F32 = mybir.dt.float32
F32R = mybir.dt.float32r
BF16 = mybir.dt.bfloat16
AF = mybir.ActivationFunctionType
GROUPS4 = [[0, 2, 4, 6], [1, 3, 5, 7]]   # gather S-shards across row-groups
GROUPS2 = [[0, 1], [2, 3], [4, 5], [6, 7]]  # exchange head halves within pair
EPS = 1e-8

_CACHE = {}


def build_nc():
    nc = bacc.Bacc("TRN2", target_bir_lowering=False, debug=False,
                   num_devices=NCORES)
    dram = {}

    def din(name, shape, dt=BF16):
        dram[name] = nc.dram_tensor(name, shape, dt, kind="ExternalInput").ap()

    # raw feature slices (feat-major, this core's 256 rows)
    din("x_text", [E, RG], F32)
    din("x_loc", [E, RG], F32)
    din("x_glob", [E, RG], F32)
    # full (replicated) projection weights, host-transposed to [in, out]
    for w in ("w_tl", "w_tg", "w_rep"):
        din(w, [E, E], F32R)
    for b in ("b_tl", "b_tg", "b_rep"):
        din(b, [E], F32)
    din("pos_l", [E], F32)
    din("pos_g", [E], F32)
    # per-MHA weights; q/k/v are this core's head-half [in, 512]
    for m in ("tl", "tg", "ff", "rt"):
        din(f"wq_{m}", [E, 512])
        din(f"wk_{m}", [E, 512])
        din(f"wv_{m}", [E, 512])
        din(f"wo_{m}", [E, E], F32R)
        din(f"bq_{m}", [512], F32)
        din(f"bk_{m}", [512], F32)
        din(f"bv_{m}", [512], F32)
        din(f"bo_{m}", [E], F32)

    out_logits = nc.dram_tensor("logits", [RG, RG], F32,
                                kind="ExternalOutput").ap()

    from contextlib import ExitStack
    with tile.TileContext(nc) as tc, ExitStack() as ctx:
        consts = ctx.enter_context(tc.tile_pool(name="consts", bufs=1))
        acts = ctx.enter_context(tc.tile_pool(name="acts", bufs=4))
        pers = ctx.enter_context(tc.tile_pool(name="pers", bufs=1))
        qps = ctx.enter_context(tc.tile_pool(name="qps", bufs=3))
        exps = ctx.enter_context(tc.tile_pool(name="exps", bufs=2))
        kpfp = ctx.enter_context(tc.tile_pool(name="kpfp", bufs=2))
        vpfp = ctx.enter_context(tc.tile_pool(name="vpfp", bufs=1))
        accs = ctx.enter_context(tc.tile_pool(name="accs", bufs=2))
        accfp = ctx.enter_context(tc.tile_pool(name="accfp", bufs=1))
        kvs = ctx.enter_context(tc.tile_pool(name="kvs", bufs=2))
        sqs = ctx.enter_context(tc.tile_pool(name="sqs", bufs=2))
        bcs = ctx.enter_context(tc.tile_pool(name="bcs", bufs=2))
        smalls = ctx.enter_context(tc.tile_pool(name="smalls", bufs=2))
        weights = ctx.enter_context(tc.tile_pool(name="weights", bufs=3))
        outs = ctx.enter_context(tc.tile_pool(name="outs", bufs=1))
        ps256 = ctx.enter_context(tc.tile_pool(name="ps256", bufs=3, space="PSUM"))
        ps512 = ctx.enter_context(tc.tile_pool(name="ps512", bufs=2, space="PSUM"))
        pssum = ctx.enter_context(tc.tile_pool(name="pssum", bufs=2, space="PSUM"))
        dram_p = ctx.enter_context(tc.tile_pool(name="dram_p", bufs=1, space="DRAM"))

        # ---------- constants ----------
        ones_cb = consts.tile([P, 1], BF16)
        nc.vector.memset(ones_cb, 1.0)
        # f32r ones for the fp32r norm path (memset can't write f32r)
        ones_cf = consts.tile([P, 1], F32)
        nc.vector.memset(ones_cf, 1.0)
        ones_col = consts.tile([P, 1], F32R)
        nc.vector.tensor_copy(ones_col, ones_cf)
        ones_rf = consts.tile([1, P], F32)
        nc.vector.memset(ones_rf, 1.0)
        ones_row = consts.tile([1, P], F32R)
        nc.vector.tensor_copy(ones_row, ones_rf)

        def load_bias_pp(name, n):
            """[n] dram -> [128, n//128] per-partition scalar layout."""
            t = consts.tile([P, n // P], F32, name=f"c_{name}")
            nc.sync.dma_start(t, dram[name].rearrange("(c p) -> p c", p=P))
            return t

        bias_pp = {}
        for nm in ("b_tl", "b_tg", "b_rep", "pos_l", "pos_g"):
            bias_pp[nm] = load_bias_pp(nm, E)
        for m in ("tl", "tg", "ff", "rt"):
            for bn in ("bq", "bk", "bv"):
                bias_pp[f"{bn}_{m}"] = load_bias_pp(f"{bn}_{m}", 512)
            bias_pp[f"bo_{m}"] = load_bias_pp(f"bo_{m}", E)

        # ---------- helpers ----------
        def load_w(name, half):
            """weight [1024, 512] (or half of [1024,1024]) -> [128,8,512]."""
            wdt = dram[name].dtype
            t = weights.tile([P, KO, 512], wdt, tag="w",
                             name=f"w_{name}_{half}",
                             padded_shape=[P, KO, 1024] if wdt == BF16 else None)
            src = dram[name]
            if src.shape[1] == E:
                src = src[:, half * 512:(half + 1) * 512]
            nc.sync.dma_start(t, src.rearrange("(ko p) c -> p ko c", p=P))
            return t

        def bcast_row(row_f32r, n):
            """[1, n] f32r -> [128, n] f32 broadcast via K=1 outer product."""
            ps = ps256.tile([P, n], F32, tag="mm", name="ps_bc")
            nc.tensor.matmul(ps, ones_row, row_f32r, start=True, stop=True)
            out = bcs.tile([P, n], F32, tag=f"bc{n}", name="bc")
            nc.any.tensor_copy(out=out, in_=ps)
            return out

        def gemm_fm(w_tiles, act, out, nco, bias=None, residual=None):
            """Feat-major GEMM: out[:, c, :] = sum_ko w[:, ko, c-chunk].T @ act[:, ko]
            w_tiles: list of [128, 8, 512] bf16 tiles covering nco*128 chans.
            act [128, 8, R] bf16; out [128, nco, R]; bias [128, nco] f32."""
            R = act.shape[2]
            for c in range(nco):
                w_sb = w_tiles[c // 4]
                cc = c % 4
                ps = ps256.tile([P, R], F32, tag="mm", name="ps_g")
                for ko in range(KO):
                    nc.tensor.matmul(ps, w_sb[:, ko, cc * P:(cc + 1) * P],
                                     act[:, ko], start=(ko == 0),
                                     stop=(ko == KO - 1))
                if bias is not None:
                    nc.vector.tensor_scalar_add(out[:, c], ps, bias[:, c:c + 1])
                    if residual is not None:
                        nc.vector.tensor_add(out[:, c], out[:, c],
                                             residual[:, c])
                elif residual is not None:
                    nc.vector.tensor_add(out[:, c], ps, residual[:, c])
                else:
                    nc.any.tensor_copy(out=out[:, c], in_=ps)

        def colsum_inv(src, nko, with_sqrt_eps=False):
            """src [128, nko, R]: per-free-column 1/||col||; returns [1, R] f32r."""
            R = src.shape[2]
            ps = pssum.tile([1, R], F32, tag="cs", name="ps_cs")
            for ko in range(nko):
                sq = sqs.tile([P, R], F32R, tag="sq", name="sq")
                nc.vector.tensor_mul(sq, src[:, ko].bitcast(F32),
                                     src[:, ko].bitcast(F32))
                nc.tensor.matmul(ps, ones_col, sq, start=(ko == 0),
                                 stop=(ko == nko - 1))
            inv = smalls.tile([1, R], F32R, tag="inv", name="inv")
            norm = smalls.tile([1, R], F32, tag="nrm", name="nrm")
            nc.scalar.sqrt(norm, ps)
            if with_sqrt_eps:
                nc.vector.tensor_scalar_max(norm, norm, EPS)
            with nc.allow_low_precision(reason="fp32r rounding intended"):
                nc.vector.reciprocal(inv, norm)
            return inv

        def attention(qp, kpf, vpf, acc_out, bv_pp):
            """qp [128,4,256] bf16; kpf [128,4(gs),4(dc),256] bf16;
            vpf [128,4(gs),2(sc),512] bf16; acc_out [128,4,256] bf16."""
            for h in range(2):
                expt = exps.tile([P, KO, RG], F32R, tag="exp", name=f"expt{h}")
                pss = pssum.tile([1, RG], F32, tag="cs", name="ps_sm")
                for s in range(8):
                    ps = ps256.tile([P, RG], F32, tag="mm", name="ps_sc")
                    for dk in range(2):
                        nc.tensor.matmul(
                            ps,
                            kpf[:, s // 2, 2 * h + dk,
                                (s % 2) * P:(s % 2 + 1) * P],
                            qp[:, 2 * h + dk],
                            start=(dk == 0), stop=(dk == 1))
                    nc.scalar.activation(expt[:, s], ps, AF.Exp, scale=0.0625)
                for s in range(8):
                    nc.tensor.matmul(pss, ones_col, expt[:, s],
                                     start=(s == 0), stop=(s == 7))
                inv = smalls.tile([1, RG], F32R, tag="inv", name="inv_sm")
                with nc.allow_low_precision(reason="fp32r rounding intended"):
                    nc.vector.reciprocal(inv, pss)
                bc = bcast_row(inv, RG)
                for dk in range(2):
                    ps = ps256.tile([P, RG], F32, tag="mm", name="ps_av")
                    for s in range(8):
                        nc.tensor.matmul(
                            ps,
                            vpf[:, s // 2, s % 2,
                                256 * h + P * dk:256 * h + P * (dk + 1)],
                            expt[:, s],
                            start=(s == 0), stop=(s == 7))
                    nc.vector.tensor_mul(acc_out[:, 2 * h + dk], ps, bc)
                    nc.vector.tensor_scalar_add(
                        acc_out[:, 2 * h + dk], acc_out[:, 2 * h + dk],
                        bv_pp[:, 2 * h + dk:2 * h + dk + 1])

        def kv_project(m, kv_src):
            """returns (kp [128,4,256] bf16, vp [128,2,512] bf16)."""
            wk = load_w(f"wk_{m}", 0)
            kp = kvs.tile([P, 4, RG], BF16, tag="kp", name=f"kp_{m}")
            gemm_fm([wk], kv_src, kp, 4, bias=bias_pp[f"bk_{m}"])
            wv = load_w(f"wv_{m}", 0)
            vp = kvs.tile([P, 2, 512], F32R, tag="vp", name=f"vp_{m}")
            for mc in range(2):
                ps = ps512.tile([P, 512], F32, tag="mm512", name="ps_vp")
                for ko in range(KO):
                    nc.tensor.matmul(ps, kv_src[:, ko, mc * P:(mc + 1) * P],
                                     wv[:, ko], start=(ko == 0),
                                     stop=(ko == KO - 1))
                nc.any.tensor_copy(out=vp[:, mc], in_=ps)
            return kp, vp

        def pack_piece(inbuf, off, sb_tile):
            if sb_tile.dtype == BF16 and inbuf.dtype != BF16:
                sb_tile = sb_tile.bitcast(F32R)
            shp = sb_tile.shape
            n = P * shp[1] * shp[2]
            nc.sync.dma_start(
                inbuf[off:off + n].rearrange("(p a b) -> p a b", p=P,
                                             a=shp[1]), sb_tile)

        def allgather(inbuf, outbuf, groups):
            nc.gpsimd.collective_compute(
                "AllGather", mybir.AluOpType.bypass,
                replica_groups=groups,
                ins=[inbuf.opt()], outs=[outbuf.opt()])

        def load_kv_full(outbuf, kp_off, vp_off, m):
            # kp piece: bf16 stored as f32r pairs (PIECE//2 f32r elems);
            # vp piece: native f32r (PIECE elems)
            kpf = kpfp.tile([P, 4, 4, RG], BF16, tag="kpf", name=f"kpf_{m}")
            vpf = vpfp.tile([P, 4, 2, 512], F32R, tag="vpf", name=f"vpf_{m}")
            for gs in range(4):
                nc.sync.dma_start(
                    kpf[:, gs].bitcast(F32R),
                    outbuf[gs, kp_off:kp_off + PIECE // 2].rearrange(
                        "(p a b) -> p a b", p=P, a=4))
                nc.sync.dma_start(
                    vpf[:, gs],
                    outbuf[gs, vp_off:vp_off + PIECE].rearrange(
                        "(p a b) -> p a b", p=P, a=2))
            return kpf, vpf

        def out_proj(m, outbuf2, acc_off, residual, out_tile):
            accf = accfp.tile([P, KO, RG], F32R, tag="accf", name=f"accf_{m}")
            for pos in range(2):
                nc.sync.dma_start(
                    accf[:, pos * 4:(pos + 1) * 4],
                    outbuf2[pos, acc_off:acc_off + PIECE].rearrange(
                        "(p a b) -> p a b", p=P, a=4))
            wo = [load_w(f"wo_{m}", 0), load_w(f"wo_{m}", 1)]
            gemm_fm(wo, accf, out_tile, 8, bias=bias_pp[f"bo_{m}"],
                    residual=residual)

        # ---------- stage 0: load + normalize ----------
        def load_raw(name):
            t = acts.tile([P, KO, RG], F32, tag="act", name=f"raw_{name}")
            nc.sync.dma_start(t, dram[name].rearrange("(ko p) r -> p ko r",
                                                      p=P))
            return t

        textT = load_raw("x_text")
        locT = load_raw("x_loc")
        globT = load_raw("x_glob")

        def normalize(raw, out, pos_pp=None):
            inv = colsum_inv(raw, KO)
            bc = bcast_row(inv, RG)
            for ko in range(KO):
                nc.vector.tensor_mul(out[:, ko], raw[:, ko], bc)
                if pos_pp is not None:
                    nc.vector.tensor_scalar_add(out[:, ko], out[:, ko],
                                                pos_pp[:, ko:ko + 1])

        # textn: f32r master (t_x GEMMs) + bf16 copy (q/k/v projections)
        textn = acts.tile([P, KO, RG], F32R, tag="act", name="textn")
        normalize(textT, textn)
        textn_bf = acts.tile([P, KO, RG], BF16, tag="actb", name="textn_bf")
        for ko in range(KO):
            nc.vector.tensor_copy(textn_bf[:, ko], textn[:, ko])
        localn = pers.tile([P, KO, RG], F32R, name="localn")
        normalize(locT, localn)
        kvl = acts.tile([P, KO, RG], BF16, tag="actb", name="kvl")
        for ko in range(KO):
            nc.vector.tensor_scalar_add(kvl[:, ko], localn[:, ko].bitcast(F32),
                                        bias_pp["pos_l"][:, ko:ko + 1])
        kvg = acts.tile([P, KO, RG], BF16, tag="actb", name="kvg")
        normalize(globT, kvg, pos_pp=bias_pp["pos_g"])

        # ---------- stage A: text projections ----------
        qp_tl = qps.tile([P, 4, RG], BF16, tag="qp", name="qp_tl")
        gemm_fm([load_w("wq_tl", 0)], textn_bf, qp_tl, 4, bias=bias_pp["bq_tl"])
        qp_tg = qps.tile([P, 4, RG], BF16, tag="qp", name="qp_tg")
        gemm_fm([load_w("wq_tg", 0)], textn_bf, qp_tg, 4, bias=bias_pp["bq_tg"])
        t_l = acts.tile([P, KO, RG], F32, tag="act", name="t_l")
        gemm_fm([load_w("w_tl", 0), load_w("w_tl", 1)], textn, t_l, 8,
                bias=bias_pp["b_tl"])
        t_g = acts.tile([P, KO, RG], F32, tag="act", name="t_g")
        gemm_fm([load_w("w_tg", 0), load_w("w_tg", 1)], textn, t_g, 8,
                bias=bias_pp["b_tg"])
        # t_r: f32r master (AG piece + fullT lhsT); bf16 copy for qp_rt
        t_r = acts.tile([P, KO, RG], F32R, tag="act", name="t_r")
        gemm_fm([load_w("w_rep", 0), load_w("w_rep", 1)], textn, t_r, 8,
                bias=bias_pp["b_rep"])
        t_r_bf = acts.tile([P, KO, RG], BF16, tag="actb", name="t_r_bf")
        for ko in range(KO):
            nc.vector.tensor_copy(t_r_bf[:, ko], t_r[:, ko])

        # ---------- stage B: tl + tg MHAs ----------
        kp_tl, vp_tl = kv_project("tl", kvl)
        kp_tg, vp_tg = kv_project("tg", kvg)
        in1 = dram_p.tile([3 * PIECE], F32R, name="in1")
        out1 = dram_p.tile([4, 3 * PIECE], F32R, name="out1")
        pack_piece(in1, 0, kp_tl)                      # PIECE//2
        pack_piece(in1, PIECE // 2, vp_tl)             # PIECE
        pack_piece(in1, 3 * PIECE // 2, kp_tg)         # PIECE//2
        pack_piece(in1, 2 * PIECE, vp_tg)              # PIECE
        allgather(in1, out1, GROUPS4)

        kpf_tl, vpf_tl = load_kv_full(out1, 0, PIECE // 2, "tl")
        acc_tl = accs.tile([P, 4, RG], F32R, tag="acc", name="acc_tl")
        attention(qp_tl, kpf_tl, vpf_tl, acc_tl, bias_pp["bv_tl"])
        kpf_tg, vpf_tg = load_kv_full(out1, 3 * PIECE // 2, 2 * PIECE, "tg")
        acc_tg = accs.tile([P, 4, RG], F32R, tag="acc", name="acc_tg")
        attention(qp_tg, kpf_tg, vpf_tg, acc_tg, bias_pp["bv_tg"])

        in2 = dram_p.tile([2 * PIECE], F32R, name="in2")
        out2 = dram_p.tile([2, 2 * PIECE], F32R, name="out2")
        pack_piece(in2, 0, acc_tl)
        pack_piece(in2, PIECE, acc_tg)
        allgather(in2, out2, GROUPS2)

        # lt / ff have residual uses -> keep f32 master + bf16 GEMM copy
        lt = acts.tile([P, KO, RG], F32, tag="act", name="lt")
        out_proj("tl", out2, 0, t_l, lt)
        gt = acts.tile([P, KO, RG], BF16, tag="actb", name="gt")
        out_proj("tg", out2, PIECE, t_g, gt)
        lt_bf = acts.tile([P, KO, RG], BF16, tag="actb", name="lt_bf")
        for ko in range(KO):
            nc.vector.tensor_copy(lt_bf[:, ko], lt[:, ko])

        # ---------- stage C: ff MHA (q=lt, kv=gt) ----------
        qp_ff = qps.tile([P, 4, RG], BF16, tag="qp", name="qp_ff")
        gemm_fm([load_w("wq_ff", 0)], lt_bf, qp_ff, 4, bias=bias_pp["bq_ff"])
        kp_ff, vp_ff = kv_project("ff", gt)
        in3 = dram_p.tile([3 * PIECE // 2], F32R, name="in3")
        out3 = dram_p.tile([4, 3 * PIECE // 2], F32R, name="out3")
        pack_piece(in3, 0, kp_ff)
        pack_piece(in3, PIECE // 2, vp_ff)
        allgather(in3, out3, GROUPS4)

        kpf_ff, vpf_ff = load_kv_full(out3, 0, PIECE // 2, "ff")
        acc_ff = accs.tile([P, 4, RG], F32R, tag="acc", name="acc_ff")
        attention(qp_ff, kpf_ff, vpf_ff, acc_ff, bias_pp["bv_ff"])
        in4 = dram_p.tile([PIECE], F32R, name="in4")
        out4 = dram_p.tile([2, PIECE], F32R, name="out4")
        pack_piece(in4, 0, acc_ff)
        allgather(in4, out4, GROUPS2)
        ff = acts.tile([P, KO, RG], BF16, tag="actb", name="ff")
        out_proj("ff", out4, 0, lt, ff)

        # ---------- stage D: rt MHA (q=t_r, kv=ff) ----------
        qp_rt = qps.tile([P, 4, RG], BF16, tag="qp", name="qp_rt")
        gemm_fm([load_w("wq_rt", 0)], t_r_bf, qp_rt, 4, bias=bias_pp["bq_rt"])
        kp_rt, vp_rt = kv_project("rt", ff)
        in5 = dram_p.tile([7 * PIECE // 2], F32R, name="in5")
        out5 = dram_p.tile([4, 7 * PIECE // 2], F32R, name="out5")
        pack_piece(in5, 0, kp_rt)                     # PIECE//2
        pack_piece(in5, PIECE // 2, vp_rt)            # PIECE
        pack_piece(in5, 3 * PIECE // 2, t_r)          # 2*PIECE
        allgather(in5, out5, GROUPS4)

        kpf_rt, vpf_rt = load_kv_full(out5, 0, PIECE // 2, "rt")
        acc_rt = accs.tile([P, 4, RG], F32R, tag="acc", name="acc_rt")
        attention(qp_rt, kpf_rt, vpf_rt, acc_rt, bias_pp["bv_rt"])
        in6 = dram_p.tile([PIECE], F32R, name="in6")
        out6 = dram_p.tile([2, PIECE], F32R, name="out6")
        pack_piece(in6, 0, acc_rt)
        allgather(in6, out6, GROUPS2)
        rt = acts.tile([P, KO, RG], F32R, tag="act", name="rt")
        out_proj("rt", out6, 0, None, rt)

        # ---------- stage E: full = rt @ t_r.T, cosine logits ----------
        fullT = acts.tile([P, KO, RG], F32, tag="act", name="fullT")
        for gs in range(4):
            trf = exps.tile([P, KO, RG], F32R, tag="exp", name=f"trf{gs}")
            nc.sync.dma_start(
                trf, out5[gs, 3 * PIECE // 2:7 * PIECE // 2].rearrange(
                    "(p a b) -> p a b", p=P, a=KO))
            for mh in range(2):
                mc = gs * 2 + mh
                ps = ps256.tile([P, RG], F32, tag="mm", name="ps_full")
                for ko in range(KO):
                    nc.tensor.matmul(ps, trf[:, ko, mh * P:(mh + 1) * P],
                                     rt[:, ko], start=(ko == 0),
                                     stop=(ko == KO - 1))
                nc.any.tensor_copy(out=fullT[:, mc], in_=ps)

        inv_full = colsum_inv(fullT, KO, with_sqrt_eps=True)
        bc_full = bcast_row(inv_full, RG)
        ffn = acts.tile([P, KO, RG], F32R, tag="act", name="ffn")
        for ko in range(KO):
            nc.vector.tensor_mul(ffn[:, ko], fullT[:, ko], bc_full)

        lg = outs.tile([P, 2, RG], F32, name="lg")
        for lc in range(2):
            ps = ps256.tile([P, RG], F32, tag="mm", name="ps_lg")
            for ko in range(KO):
                nc.tensor.matmul(ps, ffn[:, ko, lc * P:(lc + 1) * P],
                                 localn[:, ko], start=(ko == 0),
                                 stop=(ko == KO - 1))
            nc.any.tensor_copy(out=lg[:, lc], in_=ps)
        nc.sync.dma_start(out_logits.rearrange("(lc p) q -> p lc q", p=P), lg)

    nc.compile()
    return nc


def make_in_maps(local_feat, global_feat, text_feat,
                 w_tl, b_tl, w_tg, b_tg, w_rep, b_rep,
                 pos_local, pos_global, mha_params):
    """mha_params: dict m -> (wi, bi, wo, bo)."""
    f32 = np.float32
    bf16 = ml_dtypes.bfloat16
    textT = np.ascontiguousarray(text_feat.T.astype(f32))
    locT = np.ascontiguousarray(local_feat.T.astype(f32))
    globT = np.ascontiguousarray(global_feat.T.astype(f32))
    shared = {
        "w_tl": np.ascontiguousarray(w_tl.T.astype(f32)),
        "w_tg": np.ascontiguousarray(w_tg.T.astype(f32)),
        "w_rep": np.ascontiguousarray(w_rep.T.astype(f32)),
        "b_tl": b_tl.astype(f32), "b_tg": b_tg.astype(f32),
        "b_rep": b_rep.astype(f32),
        "pos_l": pos_local.astype(f32), "pos_g": pos_global.astype(f32),
    }
    per_j = {}
    for j in range(2):
        d = {}
        for m, (wi, bi, wo, bo) in mha_params.items():
            sl = slice(512 * j, 512 * (j + 1))
            d[f"wq_{m}"] = np.ascontiguousarray(wi[0 * E:1 * E][sl].T.astype(bf16))
            d[f"wk_{m}"] = np.ascontiguousarray(wi[1 * E:2 * E][sl].T.astype(bf16))
            d[f"wv_{m}"] = np.ascontiguousarray(wi[2 * E:3 * E][sl].T.astype(bf16))
            d[f"wo_{m}"] = np.ascontiguousarray(wo.T.astype(f32))
            d[f"bq_{m}"] = bi[0 * E:1 * E][sl].astype(f32)
            d[f"bk_{m}"] = bi[1 * E:2 * E][sl].astype(f32)
            d[f"bv_{m}"] = bi[2 * E:3 * E][sl].astype(f32)
            d[f"bo_{m}"] = bo.astype(f32)
        per_j[j] = d

    in_maps = []
    for c in range(NCORES):
        g, j = c // 2, c % 2
        rs = slice(RG * g, RG * (g + 1))
        m = {
            "x_text": np.ascontiguousarray(textT[:, rs]),
            "x_loc": np.ascontiguousarray(locT[:, rs]),
            "x_glob": np.ascontiguousarray(globT[:, rs]),
        }
        m.update(shared)
        m.update(per_j[j])
        in_maps.append(m)
    return in_maps


def kernel(local_feat, global_feat, text_feat,
           w_tl, b_tl, w_tg, b_tg, w_rep, b_rep,
           pos_local, pos_global,
           tl_wi, tl_bi, tl_wo, tl_bo,
           tg_wi, tg_bi, tg_wo, tg_bo,
           ff_wi, ff_bi, ff_wo, ff_bo,
           rt_wi, rt_bi, rt_wo, rt_bo,
           n_groups):
    assert int(n_groups) == 4
    if "nc" not in _CACHE:
        _CACHE["nc"] = build_nc()
    nc = _CACHE["nc"]
    mha_params = {
        "tl": (tl_wi, tl_bi, tl_wo, tl_bo),
        "tg": (tg_wi, tg_bi, tg_wo, tg_bo),
        "ff": (ff_wi, ff_bi, ff_wo, ff_bo),
        "rt": (rt_wi, rt_bi, rt_wo, rt_bo),
    }
    in_maps = make_in_maps(np.asarray(local_feat), np.asarray(global_feat),
                           np.asarray(text_feat),
                           np.asarray(w_tl), np.asarray(b_tl),
                           np.asarray(w_tg), np.asarray(b_tg),
                           np.asarray(w_rep), np.asarray(b_rep),
                           np.asarray(pos_local), np.asarray(pos_global),
                           {k: tuple(np.asarray(x) for x in v)
                            for k, v in mha_params.items()})
    res = run_bass_kernel_spmd(nc, in_maps, core_ids=list(range(NCORES)))
    _CACHE["last_results"] = res
    out = np.empty((4, RG, RG), dtype=np.float32)
    for g in range(4):
        out[g] = res.results[2 * g]["logits"]
    return out
